# revision 48
# baseline (speedup 1.0000x reference)
"""Data-dependent RBF kernel for Trainium2, data-parallel over batch B=8.

Per core b:
  sigma[n]   = 0.1 + 9.9*sigmoid(MLP(emb[n]))           (tiny MLP)
  out[n, m]  = exp(-((z0[m]-mu0[n])^2 + (z1[m]-mu1[n])^2) / (2 sigma[n]^2))

All layout work is done on the HOST, so the device program is just
matmuls + activations + straight-line contiguous DMAs:

- embT, MLP weights, and the distance-expansion rows for z (moving) and
  mu (stationary) are prepacked into fp16 arrays in numpy and DMA'd in
  with fully contiguous 2-4KB descriptors (no on-chip transposes,
  splits, or scatter DMAs).
- d2 expansion: psum[n,m] = sum_k aug[k,n] zr[k,m] with K=10 fp16 rows:
  hi/lo cross products per coordinate plus split -|z|^2 and -|mu|^2 rows
  (accurate to ~1e-4), so the ACT Exp needs no bias operand at all.
- the MLP runs as two 512-row chunks pipelined across PE and ACT. PSUM
  is exactly 2 x [128, 2048] ring slots; the MLP psums alias into them
  via slices, arranged so every (tile-granular) Tile dep coincides with
  a real data dep and ring reuse orders the main loop for free.
- sigmoid is computed as 0.5*(1+tanh(x/2)); tanh lives in BOTH the gelu
  and exp ACT table sets, so the single table switch (gelu->exp) starts
  right after the last gelu and both tanhs run from the exp set. The
  warm-up exp reads g2B's output so walrus cannot hoist it earlier
  (which would thrash the table sets).
- mm3 (w3^T h2) uses h2 128-column slabs as the *stationary* operand so
  sigma lands directly in [128-partition, 8] layout -- no transposes
  anywhere in the program.
- main loop: per 128-row tile, 4x 512-col fp16 matmuls into a [128,2048]
  PSUM slot, one 2048-wide ACT Exp (per-partition scale=1/(2s^2),
  bias=0) writing fp16, one contiguous 512KB store; the last tile is
  split in half so the final DMA flush is half the bytes. The steady
  state is jointly bounded by the ACT engine (~2.06us/tile) and the
  output DMA wire (~245GB/s aggregate); the fp16 output (host-upcast to
  f32) halves the store bytes vs f32 at ~1e-4 relative error.
"""

import numpy as np

_B, _N, _M, _P, _E, _H, _H2 = 8, 1024, 2048, 2, 256, 32, 16
_KR = 10  # distance-expansion rows (incl. -r_z and -r_mu hi/lo)
_NT = _N // 128  # 8 row tiles per core

_SQ2 = 1.4142135623730951

_CACHE = {}
LAST_RESULTS = None


def _install_drain_patch():
    """walrus in this container allows at most 2 sync-wait commands per
    instruction, but TileContext's final drain aggregates a wait per live
    Tile semaphore onto one Drain. Emit one Drain per wait instead."""
    import concourse.tile as _tile
    from concourse.vector_clock import ScopedClock
    from concourse import mybir as _mybir

    if getattr(_tile.TileContext, "_drain_waits_split", False):
        return

    def _split_drain_and_barrier(self, tick_clock, wait_clock):
        nc = self.nc
        probe = _mybir.InstDrain(name="probe-drain-waits")
        probe.engine = _mybir.EngineType.SP
        wait_clock.add_sem_waits(probe, ScopedClock({None: tick_clock.global_clock}))
        si = probe.sync_info
        waits = list(si.on_wait) if si is not None else []

        assert self.sems is not None
        by_name = {h.name: h for h in self.sems.allocated().values()}

        if not waits:
            nc.sync.drain()
        for w in waits:
            nc.sync.drain().wait_op(by_name[w.ant_name], w.wait_value, "sem-ge")

        nc.all_engine_barrier()
        popped = nc._tile_sem_poison_stack.pop()
        assert popped is self._sem_poison
        nc.clear_and_free_semaphores(list(self.sems.allocated().values()))

    _tile.TileContext._drain_and_barrier = _split_drain_and_barrier
    _tile.TileContext._drain_waits_split = True


def _install_wait_split_patch():
    """walrus in this container rejects instructions carrying more than 2
    sync-wait commands (and matmuls more than ~1). Tile's sem assignment can
    attach several waits to one instruction, so post-process the serialized
    BIR: excess waits move onto EventSemaphore instructions inserted just
    before the instruction on the same engine (engines execute in program
    order, so this is equivalent)."""
    import orjson
    import concourse.bass as bass

    if getattr(bass.Bass, "_wait_split_patched", False):
        return
    orig = bass.Bass.to_json_bytes
    MAXW = 1

    def to_json_bytes(self):
        j = orjson.loads(orig(self))
        cnt = 0
        for f in j.get("functions", []):
            for blk in f.get("blocks", []):
                insts = blk.get("instructions", [])
                out = []
                changed = False
                for inst in insts:
                    si = inst.get("sync_info")
                    waits = (si or {}).get("on_wait") or []
                    if len(waits) > MAXW:
                        changed = True
                        extra, keep = waits[:-MAXW], waits[-MAXW:]
                        for k in range(0, len(extra), MAXW):
                            cnt += 1
                            out.append(
                                {
                                    "debug": inst.get("debug"),
                                    "engine": inst["engine"],
                                    "ins": [],
                                    "outs": [],
                                    "name": f"waitsplit-{cnt}",
                                    "opcode": "EventSemaphore",
                                    "sync_info": {
                                        "on_update": [],
                                        "on_wait": extra[k : k + MAXW],
                                    },
                                }
                            )
                        si["on_wait"] = keep
                    out.append(inst)
                if changed:
                    blk["instructions"] = out
        return orjson.dumps(j)

    bass.Bass.to_json_bytes = to_json_bytes
    bass.Bass._wait_split_patched = True


def _build_program():
    import concourse.bass as bass
    import concourse.tile as tile
    from concourse import mybir

    f32 = mybir.dt.float32
    f16 = mybir.dt.float16
    FT = mybir.ActivationFunctionType

    nc = bass.Bass(enable_asserts=False, detect_race_conditions=False)

    ehT_d = nc.dram_tensor("ehT", [128, 2, _N], f16, kind="ExternalInput")
    pk16_d = nc.dram_tensor("pk16", [128, 96], f16, kind="ExternalInput")
    pkf_d = nc.dram_tensor("pkf", [128, 16], f32, kind="ExternalInput")
    zr_d = nc.dram_tensor("zr", [_KR, _M], f16, kind="ExternalInput")
    aug_d = nc.dram_tensor("aug", [_KR, _N], f16, kind="ExternalInput")
    out_d = nc.dram_tensor("out", [_N, _M], f16, kind="ExternalOutput")

    with tile.TileContext(nc) as tc:
        with (
            tc.tile_pool(name="singles", bufs=1) as singles,
            tc.tile_pool(name="outp", bufs=6) as outp,
        ):
            # ---- DMA issues + gelu table load, all up front -------------
            one11 = singles.tile([1, 1], f32)
            nc.vector.memset(one11, 1.0)
            warmg = singles.tile([1, 1], f32)
            nc.scalar.activation(out=warmg, in_=one11, func=FT.Gelu)

            # Everything latency-critical goes on the two HWDGE queues
            # (sync + ACT sequencer) in parallel; the gpsimd SWDGE path has
            # ~5us latency. ehT is split in four quarter-DMAs spread over
            # both queues so each mm1 operand lands as early as possible.
            A, B = slice(0, 512), slice(512, 1024)
            # ONE big ehT DMA: 4KB-contiguous per-partition descriptors;
            # input DMA tops out ~150-190GB/s regardless of how it's split,
            # so keep it simple with zero queue contention.
            ehT = singles.tile([128, 2, _N], f16)
            nc.sync.dma_start(out=ehT, in_=ehT_d[:, :, :])
            pk16 = singles.tile([128, 96], f16)
            nc.scalar.dma_start(out=pk16, in_=pk16_d[:, :])
            pkf = singles.tile([128, 16], f32)
            nc.scalar.dma_start(out=pkf, in_=pkf_d[:, :])
            zr = singles.tile([_KR, _M], f16)
            nc.sync.dma_start(out=zr, in_=zr_d[:, :])
            aug = singles.tile([_KR, _N], f16)

            # per-chunk tiles: Tile WAR deps are tile-granular, so chunk B
            # writes must not share a tile with chunk A's values.
            h1c = [singles.tile([_H, 512], f16, name=f"h1c{c}") for c in range(2)]
            h2c = [singles.tile([_H2, 512], f16, name=f"h2c{c}") for c in range(2)]
            thc = [singles.tile([128, 4], f32, name=f"thc{c}") for c in range(2)]
            sgc = [singles.tile([128, 4], f32, name=f"sgc{c}") for c in range(2)]
            t2c = [singles.tile([128, 4], f32, name=f"t2c{c}") for c in range(2)]
            invc = [singles.tile([128, 4], f32, name=f"invc{c}") for c in range(2)]

            def tail(c):
                # sqrt(2)*sigma = 5.05*sqrt2 + 4.95*sqrt2 * th
                nc.vector.tensor_scalar(
                    out=sgc[c],
                    in0=thc[c],
                    scalar1=4.95 * _SQ2,
                    scalar2=5.05 * _SQ2,
                    op0=mybir.AluOpType.mult,
                    op1=mybir.AluOpType.add,
                )
                nc.vector.tensor_mul(out=t2c[c], in0=sgc[c], in1=sgc[c])
                nc.vector.reciprocal(out=invc[c], in_=t2c[c])

            # ---- MLP, two 512-row chunks pipelined across PE/ACT --------
            # PSUM is just 2 x [128, 2048] ring slots; the MLP psums alias
            # into them via slices. Tile dependency tracking is TILE-
            # granular, so the aliasing is arranged such that every
            # tile-level dep coincides with a real data dep:
            #   slot0: ph1 (cols 0:1024, written by mm1, read by g1A/g1B)
            #          + ph2B (cols 1024:1536, w: mm2B, r: g2B)
            #   slot1: pt (cols 0:8, w: mm3A/B, r: tanhA/B)
            #          + ph2A (cols 1024:1536, w: mm2A, r: g2A)
            # Ring reuse then puts pd_t0 (slot0) after g2B and pd_t1
            # (slot1) after tanhB -- exactly the true ordering.
            with tc.tile_pool(name="pmain", bufs=2, space="PSUM") as pmain:
                slot0 = pmain.tile([128, _M], f32, tag="pd")
                slot1 = pmain.tile([128, _M], f32, tag="pd")
                ph1 = slot0[0:_H, 0:_N]
                ph2c = [slot1[0:_H2, _N : _N + 512], slot0[0:_H2, _N : _N + 512]]
                pt = slot1[:, 0:8]

                # mm1 in chunk-arrival order
                for k, sl in ((0, A), (0, B), (1, A), (1, B)):
                    nc.tensor.matmul(
                        ph1[:, sl],
                        pk16[:, k * 32 : (k + 1) * 32],
                        ehT[:, k, sl],
                        start=(k == 0),
                        stop=(k == 1),
                    )
                nc.scalar.activation(
                    out=h1c[0], in_=ph1[:, A], func=FT.Gelu,
                    bias=pkf[0:_H, 0:1], scale=1.0,
                )
                # aug is only needed by the main-loop matmuls; issuing its
                # DMA here keeps its descriptors from stealing DMA-engine
                # slots while ehT streams in.
                nc.scalar.dma_start(out=aug, in_=aug_d[:, :])
                nc.tensor.matmul(
                    ph2c[0], pk16[0:_H, 64:80], h1c[0], start=True, stop=True
                )
                nc.scalar.activation(
                    out=h1c[1], in_=ph1[:, B], func=FT.Gelu,
                    bias=pkf[0:_H, 0:1], scale=1.0,
                )
                nc.scalar.activation(
                    out=h2c[0], in_=ph2c[0], func=FT.Gelu,
                    bias=pkf[0:_H2, 1:2], scale=1.0,
                )
                nc.tensor.matmul(
                    ph2c[1], pk16[0:_H, 64:80], h1c[1], start=True, stop=True
                )
                # mm3 with h2 slabs stationary: sigma pre-activation lands
                # directly in [128, 8] partition layout; sigmoid(x) =
                # 0.5*(1+tanh(x/2)) -- tanh is in the gelu table set, so no
                # table switch happens anywhere in the MLP.
                for j in range(4):
                    nc.tensor.matmul(
                        pt[:, j : j + 1],
                        h2c[0][:, j * 128 : (j + 1) * 128],
                        pk16[0:_H2, 80:81],
                        start=True,
                        stop=True,
                    )
                nc.scalar.activation(
                    out=h2c[1], in_=ph2c[1], func=FT.Gelu,
                    bias=pkf[0:_H2, 1:2], scale=1.0,
                )
                for j in range(4):
                    nc.tensor.matmul(
                        pt[:, 4 + j : 5 + j],
                        h2c[1][:, j * 128 : (j + 1) * 128],
                        pk16[0:_H2, 80:81],
                        start=True,
                        stop=True,
                    )
                # g2B is the LAST gelu-set user: the exp table switch
                # starts immediately after it. tanh is in the exp set too,
                # so both tanhs run post-switch. warme reads h2c[1] (g2B's
                # output) so walrus can't hoist it above the gelus.
                warme = singles.tile([1, 1], f32)
                nc.scalar.activation(
                    out=warme, in_=h2c[1][0:1, 0:1], func=FT.Exp
                )
                nc.scalar.activation(
                    out=thc[0], in_=pt[:, 0:4], func=FT.Tanh,
                    bias=pkf[:, 2:3], scale=0.5,
                )
                tail(0)
                nc.scalar.activation(
                    out=thc[1], in_=pt[:, 4:8], func=FT.Tanh,
                    bias=pkf[:, 2:3], scale=0.5,
                )
                tail(1)

                for t in range(_NT):
                    pd = pmain.tile([128, _M], f32, tag="pd")
                    for q in range(4):
                        sl = slice(q * 512, (q + 1) * 512)
                        nc.tensor.matmul(
                            pd[:, sl],
                            aug[:, t * 128 : (t + 1) * 128],
                            zr[:, sl],
                            start=True,
                            stop=True,
                        )
                    c, tc_ = t // 4, t % 4
                    ot = outp.tile([128, _M], f16, tag="ot")
                    rows = slice(t * 128, (t + 1) * 128)
                    if t < _NT - 1:
                        nc.scalar.activation(
                            out=ot,
                            in_=pd,
                            func=FT.Exp,
                            scale=invc[c][:, tc_ : tc_ + 1],
                            bias=0.0,
                        )
                        nc.sync.dma_start(out=out_d[rows, :], in_=ot)
                    else:
                        # split the first tile (output wire starts ~1.3us
                        # earlier) and the last (final flush is half the
                        # bytes) into two EXP+store halves
                        for hh in range(2):
                            cols = slice(hh * 1024, (hh + 1) * 1024)
                            nc.scalar.activation(
                                out=ot[:, cols],
                                in_=pd[:, cols],
                                func=FT.Exp,
                                scale=invc[c][:, tc_ : tc_ + 1],
                                bias=0.0,
                            )
                            nc.sync.dma_start(
                                out=out_d[rows, cols], in_=ot[:, cols]
                            )

    return nc


def _host_pack(z, mu, embeddings, w1, b1, w2, b2, w3, b3):
    """Build per-core prepacked fp16/f32 input arrays."""
    f32 = np.float32
    f16 = np.float16

    def split(x):
        hi = x.astype(f16)
        lo = (x - hi.astype(f32)).astype(f16)
        return hi, lo

    z = z.astype(f32)
    z0, z1 = z[:, 0], z[:, 1]
    z0h, z0l = split(z0)
    z1h, z1l = split(z1)
    rz = z0 * z0 + z1 * z1
    nr1 = (-rz).astype(f16)
    nr2 = (-rz - nr1.astype(f32)).astype(f16)
    onesM = np.ones(_M, dtype=f16)
    zr = np.ascontiguousarray(
        np.stack([z0h, z0l, z0h, z1h, z1l, z1h, nr1, nr2, onesM, onesM])
    )

    pk16 = np.zeros((128, 96), dtype=f16)
    w1 = w1.astype(f32)
    pk16[:, 0:32] = w1[0:128].astype(f16)
    pk16[:, 32:64] = w1[128:256].astype(f16)
    pk16[0:_H, 64:80] = w2.astype(f16)
    pk16[0:_H2, 80] = w3.reshape(-1).astype(f16)

    cores = []
    for c in range(_B):
        mu_c = mu[c].astype(f32)
        a0 = 2.0 * mu_c[:, 0]
        a1 = 2.0 * mu_c[:, 1]
        a0h, a0l = split(a0)
        a1h, a1l = split(a1)
        ones = np.ones(_N, dtype=f16)
        rmu = (mu_c * mu_c).sum(axis=-1)
        rmh = (-rmu).astype(f16)
        rml = (-rmu - rmh.astype(f32)).astype(f16)
        aug = np.ascontiguousarray(
            np.stack([a0h, a0h, a0l, a1h, a1h, a1l, ones, ones, rmh, rml])
        )

        pkf = np.zeros((128, 16), dtype=f32)
        pkf[0:_H, 0] = b1.astype(f32)
        pkf[0:_H2, 1] = b2.astype(f32)
        pkf[:, 2] = 0.5 * float(b3.reshape(-1)[0])

        ehT = np.ascontiguousarray(
            embeddings[c].astype(f32).T.reshape(2, 128, _N).transpose(1, 0, 2)
        ).astype(f16)

        cores.append(
            {
                "ehT": ehT,
                "pk16": pk16,
                "pkf": pkf,
                "zr": zr,
                "aug": aug,
            }
        )
    return cores


def kernel(z, mu, embeddings, w1, b1, w2, b2, w3, b3):
    global LAST_RESULTS
    from concourse.bass_utils import run_bass_kernel_spmd

    _install_drain_patch()
    _install_wait_split_patch()
    if "nc" not in _CACHE:
        _CACHE["nc"] = _build_program()
    nc = _CACHE["nc"]

    in_maps = _host_pack(z, mu, embeddings, w1, b1, w2, b2, w3, b3)
    res = run_bass_kernel_spmd(nc, in_maps, list(range(_B)))
    LAST_RESULTS = res
    return np.stack(
        [res.results[c]["out"].astype(np.float32) for c in range(_B)], axis=0
    )


# revision 50
# speedup vs baseline: 1.0474x; 1.0474x over previous
"""Data-dependent RBF kernel for Trainium2, data-parallel over batch B=8.

Per core b:
  sigma[n]   = 0.1 + 9.9*sigmoid(MLP(emb[n]))           (tiny MLP)
  out[n, m]  = exp(-((z0[m]-mu0[n])^2 + (z1[m]-mu1[n])^2) / (2 sigma[n]^2))

All layout work is done on the HOST, so the device program is just
matmuls + activations + straight-line contiguous DMAs:

- embT, MLP weights, and the distance-expansion rows for z (moving) and
  mu (stationary) are prepacked into fp16 arrays in numpy and DMA'd in
  with fully contiguous 2-4KB descriptors (no on-chip transposes,
  splits, or scatter DMAs).
- d2 expansion: psum[n,m] = sum_k aug[k,n] zr[k,m] with K=10 fp16 rows:
  hi/lo cross products per coordinate plus split -|z|^2 and -|mu|^2 rows
  (accurate to ~1e-4), so the ACT Exp needs no bias operand at all.
- the MLP runs as two 512-row chunks pipelined across PE and ACT. PSUM
  is exactly 2 x [128, 2048] ring slots; the MLP psums alias into them
  via slices, arranged so every (tile-granular) Tile dep coincides with
  a real data dep and ring reuse orders the main loop for free.
- sigmoid is computed as 0.5*(1+tanh(x/2)); tanh lives in BOTH the gelu
  and exp ACT table sets, so the single table switch (gelu->exp) starts
  right after the last gelu and both tanhs run from the exp set. The
  warm-up exp reads g2B's output so walrus cannot hoist it earlier
  (which would thrash the table sets).
- mm3 (w3^T h2) uses h2 128-column slabs as the *stationary* operand so
  sigma lands directly in [128-partition, 8] layout -- no transposes
  anywhere in the program.
- main loop: per 128-row tile, 4x 512-col fp16 matmuls into a [128,2048]
  PSUM slot, one 2048-wide ACT Exp (per-partition scale=1/(2s^2),
  bias=0) writing fp16, one contiguous 512KB store; the last tile is
  split in half so the final DMA flush is half the bytes. The steady
  state is jointly bounded by the ACT engine (~2.06us/tile) and the
  output DMA wire (~245GB/s aggregate); the fp16 output (host-upcast to
  f32) halves the store bytes vs f32 at ~1e-4 relative error.
"""

import numpy as np

_B, _N, _M, _P, _E, _H, _H2 = 8, 1024, 2048, 2, 256, 32, 16
_KR = 10  # distance-expansion rows (incl. -r_z and -r_mu hi/lo)
_NT = _N // 128  # 8 row tiles per core

_SQ2 = 1.4142135623730951

_CACHE = {}
LAST_RESULTS = None


def _install_drain_patch():
    """walrus in this container allows at most 2 sync-wait commands per
    instruction, but TileContext's final drain aggregates a wait per live
    Tile semaphore onto one Drain. Emit one Drain per wait instead."""
    import concourse.tile as _tile
    from concourse.vector_clock import ScopedClock
    from concourse import mybir as _mybir

    if getattr(_tile.TileContext, "_drain_waits_split", False):
        return

    def _split_drain_and_barrier(self, tick_clock, wait_clock):
        nc = self.nc
        probe = _mybir.InstDrain(name="probe-drain-waits")
        probe.engine = _mybir.EngineType.SP
        wait_clock.add_sem_waits(probe, ScopedClock({None: tick_clock.global_clock}))
        si = probe.sync_info
        waits = list(si.on_wait) if si is not None else []

        assert self.sems is not None
        by_name = {h.name: h for h in self.sems.allocated().values()}

        if not waits:
            nc.sync.drain()
        for w in waits:
            nc.sync.drain().wait_op(by_name[w.ant_name], w.wait_value, "sem-ge")

        nc.all_engine_barrier()
        popped = nc._tile_sem_poison_stack.pop()
        assert popped is self._sem_poison
        nc.clear_and_free_semaphores(list(self.sems.allocated().values()))

    _tile.TileContext._drain_and_barrier = _split_drain_and_barrier
    _tile.TileContext._drain_waits_split = True


def _install_wait_split_patch():
    """walrus in this container rejects instructions carrying more than 2
    sync-wait commands (and matmuls more than ~1). Tile's sem assignment can
    attach several waits to one instruction, so post-process the serialized
    BIR: excess waits move onto EventSemaphore instructions inserted just
    before the instruction on the same engine (engines execute in program
    order, so this is equivalent)."""
    import orjson
    import concourse.bass as bass

    if getattr(bass.Bass, "_wait_split_patched", False):
        return
    orig = bass.Bass.to_json_bytes
    MAXW = 1

    def to_json_bytes(self):
        j = orjson.loads(orig(self))
        cnt = 0
        for f in j.get("functions", []):
            for blk in f.get("blocks", []):
                insts = blk.get("instructions", [])
                out = []
                changed = False
                for inst in insts:
                    si = inst.get("sync_info")
                    waits = (si or {}).get("on_wait") or []
                    if len(waits) > MAXW:
                        changed = True
                        extra, keep = waits[:-MAXW], waits[-MAXW:]
                        for k in range(0, len(extra), MAXW):
                            cnt += 1
                            out.append(
                                {
                                    "debug": inst.get("debug"),
                                    "engine": inst["engine"],
                                    "ins": [],
                                    "outs": [],
                                    "name": f"waitsplit-{cnt}",
                                    "opcode": "EventSemaphore",
                                    "sync_info": {
                                        "on_update": [],
                                        "on_wait": extra[k : k + MAXW],
                                    },
                                }
                            )
                        si["on_wait"] = keep
                    out.append(inst)
                if changed:
                    blk["instructions"] = out
        return orjson.dumps(j)

    bass.Bass.to_json_bytes = to_json_bytes
    bass.Bass._wait_split_patched = True


def _build_program():
    import concourse.bass as bass
    import concourse.tile as tile
    from concourse import mybir

    f32 = mybir.dt.float32
    f16 = mybir.dt.float16
    FT = mybir.ActivationFunctionType

    nc = bass.Bass(enable_asserts=False, detect_race_conditions=False)

    ehT_d = nc.dram_tensor("ehT", [128, 2, 2, 512], f16, kind="ExternalInput")
    pk16_d = nc.dram_tensor("pk16", [128, 96], f16, kind="ExternalInput")
    pkf_d = nc.dram_tensor("pkf", [128, 16], f32, kind="ExternalInput")
    zr_d = nc.dram_tensor("zr", [_KR, _M], f16, kind="ExternalInput")
    aug_d = nc.dram_tensor("aug", [_KR, _N], f16, kind="ExternalInput")
    out_d = nc.dram_tensor("out", [_N, _M], f16, kind="ExternalOutput")

    with tile.TileContext(nc) as tc:
        with (
            tc.tile_pool(name="singles", bufs=1) as singles,
            tc.tile_pool(name="outp", bufs=6) as outp,
        ):
            # ---- DMA issues + gelu table load, all up front -------------
            one11 = singles.tile([1, 1], f32)
            nc.vector.memset(one11, 1.0)
            warmg = singles.tile([1, 1], f32)
            nc.scalar.activation(out=warmg, in_=one11, func=FT.Gelu)

            # Everything latency-critical goes on the two HWDGE queues
            # (sync + ACT sequencer) in parallel; the gpsimd SWDGE path has
            # ~5us latency. ehT is split in four quarter-DMAs spread over
            # both queues so each mm1 operand lands as early as possible.
            A, B = slice(0, 512), slice(512, 1024)
            # ehT is host-packed as [p, nhalf, k, n512] so each N-half is
            # 2KB-contiguous per partition: the first DMA (256KB) alone
            # carries BOTH e-chunks for rows 0:511, unblocking the whole
            # chunk-A MLP while the B half is still in flight.
            ehTh = [
                singles.tile([128, 2, 512], f16, name=f"ehTh{h}")
                for h in range(2)
            ]
            nc.sync.dma_start(out=ehTh[0], in_=ehT_d[:, 0])
            nc.sync.dma_start(out=ehTh[1], in_=ehT_d[:, 1])
            pk16 = singles.tile([128, 96], f16)
            nc.scalar.dma_start(out=pk16, in_=pk16_d[:, :])
            pkf = singles.tile([128, 16], f32)
            nc.scalar.dma_start(out=pkf, in_=pkf_d[:, :])
            zr = singles.tile([_KR, _M], f16)
            nc.sync.dma_start(out=zr, in_=zr_d[:, :])
            aug = singles.tile([_KR, _N], f16)

            # per-chunk tiles: Tile WAR deps are tile-granular, so chunk B
            # writes must not share a tile with chunk A's values.
            h1c = [singles.tile([_H, 512], f16, name=f"h1c{c}") for c in range(2)]
            h2c = [singles.tile([_H2, 512], f16, name=f"h2c{c}") for c in range(2)]
            thc = [singles.tile([128, 4], f32, name=f"thc{c}") for c in range(2)]
            sgc = [singles.tile([128, 4], f32, name=f"sgc{c}") for c in range(2)]
            t2c = [singles.tile([128, 4], f32, name=f"t2c{c}") for c in range(2)]
            invc = [singles.tile([128, 4], f32, name=f"invc{c}") for c in range(2)]

            def tail(c):
                # sqrt(2)*sigma = 5.05*sqrt2 + 4.95*sqrt2 * th
                nc.vector.tensor_scalar(
                    out=sgc[c],
                    in0=thc[c],
                    scalar1=4.95 * _SQ2,
                    scalar2=5.05 * _SQ2,
                    op0=mybir.AluOpType.mult,
                    op1=mybir.AluOpType.add,
                )
                nc.vector.tensor_mul(out=t2c[c], in0=sgc[c], in1=sgc[c])
                nc.vector.reciprocal(out=invc[c], in_=t2c[c])

            # ---- MLP, two 512-row chunks pipelined across PE/ACT --------
            # PSUM is just 2 x [128, 2048] ring slots; the MLP psums alias
            # into them via slices. Tile dependency tracking is TILE-
            # granular, so the aliasing is arranged such that every
            # tile-level dep coincides with a real data dep:
            #   slot0: ph1 (cols 0:1024, written by mm1, read by g1A/g1B)
            #          + ph2B (cols 1024:1536, w: mm2B, r: g2B)
            #   slot1: pt (cols 0:8, w: mm3A/B, r: tanhA/B)
            #          + ph2A (cols 1024:1536, w: mm2A, r: g2A)
            # Ring reuse then puts pd_t0 (slot0) after g2B and pd_t1
            # (slot1) after tanhB -- exactly the true ordering.
            with tc.tile_pool(name="pmain", bufs=2, space="PSUM") as pmain:
                slot0 = pmain.tile([128, _M], f32, tag="pd")
                slot1 = pmain.tile([128, _M], f32, tag="pd")
                ph1 = slot0[0:_H, 0:_N]
                ph2c = [slot1[0:_H2, _N : _N + 512], slot0[0:_H2, _N : _N + 512]]
                pt = slot1[:, 0:8]

                # mm1 in chunk-arrival order: the A half (both
                # e-chunks) lands first, so chunk A finishes accumulation
                # ~1.5us before the B data even arrives
                for h, sl in ((0, A), (1, B)):
                    for k in range(2):
                        nc.tensor.matmul(
                            ph1[:, sl],
                            pk16[:, k * 32 : (k + 1) * 32],
                            ehTh[h][:, k, :],
                            start=(k == 0),
                            stop=(k == 1),
                        )
                nc.scalar.activation(
                    out=h1c[0], in_=ph1[:, A], func=FT.Gelu,
                    bias=pkf[0:_H, 0:1], scale=1.0,
                )
                # aug is only needed by the main-loop matmuls; issuing its
                # DMA here keeps its descriptors from stealing DMA-engine
                # slots while ehT streams in.
                nc.scalar.dma_start(out=aug, in_=aug_d[:, :])
                nc.tensor.matmul(
                    ph2c[0], pk16[0:_H, 64:80], h1c[0], start=True, stop=True
                )
                nc.scalar.activation(
                    out=h1c[1], in_=ph1[:, B], func=FT.Gelu,
                    bias=pkf[0:_H, 0:1], scale=1.0,
                )
                nc.scalar.activation(
                    out=h2c[0], in_=ph2c[0], func=FT.Gelu,
                    bias=pkf[0:_H2, 1:2], scale=1.0,
                )
                nc.tensor.matmul(
                    ph2c[1], pk16[0:_H, 64:80], h1c[1], start=True, stop=True
                )
                # mm3 with h2 slabs stationary: sigma pre-activation lands
                # directly in [128, 8] partition layout; sigmoid(x) =
                # 0.5*(1+tanh(x/2)) -- tanh is in the gelu table set, so no
                # table switch happens anywhere in the MLP.
                for j in range(4):
                    nc.tensor.matmul(
                        pt[:, j : j + 1],
                        h2c[0][:, j * 128 : (j + 1) * 128],
                        pk16[0:_H2, 80:81],
                        start=True,
                        stop=True,
                    )
                nc.scalar.activation(
                    out=h2c[1], in_=ph2c[1], func=FT.Gelu,
                    bias=pkf[0:_H2, 1:2], scale=1.0,
                )
                for j in range(4):
                    nc.tensor.matmul(
                        pt[:, 4 + j : 5 + j],
                        h2c[1][:, j * 128 : (j + 1) * 128],
                        pk16[0:_H2, 80:81],
                        start=True,
                        stop=True,
                    )
                # g2B is the LAST gelu-set user: the exp table switch
                # starts immediately after it. tanh is in the exp set too,
                # so both tanhs run post-switch. warme reads h2c[1] (g2B's
                # output) so walrus can't hoist it above the gelus.
                warme = singles.tile([1, 1], f32)
                nc.scalar.activation(
                    out=warme, in_=h2c[1][0:1, 0:1], func=FT.Exp
                )
                nc.scalar.activation(
                    out=thc[0], in_=pt[:, 0:4], func=FT.Tanh,
                    bias=pkf[:, 2:3], scale=0.5,
                )
                tail(0)
                nc.scalar.activation(
                    out=thc[1], in_=pt[:, 4:8], func=FT.Tanh,
                    bias=pkf[:, 2:3], scale=0.5,
                )
                tail(1)

                for t in range(_NT):
                    pd = pmain.tile([128, _M], f32, tag="pd")
                    for q in range(4):
                        sl = slice(q * 512, (q + 1) * 512)
                        nc.tensor.matmul(
                            pd[:, sl],
                            aug[:, t * 128 : (t + 1) * 128],
                            zr[:, sl],
                            start=True,
                            stop=True,
                        )
                    c, tc_ = t // 4, t % 4
                    ot = outp.tile([128, _M], f16, tag="ot")
                    rows = slice(t * 128, (t + 1) * 128)
                    if t < _NT - 1:
                        nc.scalar.activation(
                            out=ot,
                            in_=pd,
                            func=FT.Exp,
                            scale=invc[c][:, tc_ : tc_ + 1],
                            bias=0.0,
                        )
                        nc.sync.dma_start(out=out_d[rows, :], in_=ot)
                    else:
                        # split the first tile (output wire starts ~1.3us
                        # earlier) and the last (final flush is half the
                        # bytes) into two EXP+store halves
                        for hh in range(2):
                            cols = slice(hh * 1024, (hh + 1) * 1024)
                            nc.scalar.activation(
                                out=ot[:, cols],
                                in_=pd[:, cols],
                                func=FT.Exp,
                                scale=invc[c][:, tc_ : tc_ + 1],
                                bias=0.0,
                            )
                            nc.sync.dma_start(
                                out=out_d[rows, cols], in_=ot[:, cols]
                            )

    return nc


def _host_pack(z, mu, embeddings, w1, b1, w2, b2, w3, b3):
    """Build per-core prepacked fp16/f32 input arrays."""
    f32 = np.float32
    f16 = np.float16

    def split(x):
        hi = x.astype(f16)
        lo = (x - hi.astype(f32)).astype(f16)
        return hi, lo

    z = z.astype(f32)
    z0, z1 = z[:, 0], z[:, 1]
    z0h, z0l = split(z0)
    z1h, z1l = split(z1)
    rz = z0 * z0 + z1 * z1
    nr1 = (-rz).astype(f16)
    nr2 = (-rz - nr1.astype(f32)).astype(f16)
    onesM = np.ones(_M, dtype=f16)
    zr = np.ascontiguousarray(
        np.stack([z0h, z0l, z0h, z1h, z1l, z1h, nr1, nr2, onesM, onesM])
    )

    pk16 = np.zeros((128, 96), dtype=f16)
    w1 = w1.astype(f32)
    pk16[:, 0:32] = w1[0:128].astype(f16)
    pk16[:, 32:64] = w1[128:256].astype(f16)
    pk16[0:_H, 64:80] = w2.astype(f16)
    pk16[0:_H2, 80] = w3.reshape(-1).astype(f16)

    cores = []
    for c in range(_B):
        mu_c = mu[c].astype(f32)
        a0 = 2.0 * mu_c[:, 0]
        a1 = 2.0 * mu_c[:, 1]
        a0h, a0l = split(a0)
        a1h, a1l = split(a1)
        ones = np.ones(_N, dtype=f16)
        rmu = (mu_c * mu_c).sum(axis=-1)
        rmh = (-rmu).astype(f16)
        rml = (-rmu - rmh.astype(f32)).astype(f16)
        aug = np.ascontiguousarray(
            np.stack([a0h, a0h, a0l, a1h, a1h, a1l, ones, ones, rmh, rml])
        )

        pkf = np.zeros((128, 16), dtype=f32)
        pkf[0:_H, 0] = b1.astype(f32)
        pkf[0:_H2, 1] = b2.astype(f32)
        pkf[:, 2] = 0.5 * float(b3.reshape(-1)[0])

        ehT = np.ascontiguousarray(
            embeddings[c].astype(f32).T.reshape(2, 128, 2, 512)
            .transpose(1, 2, 0, 3)
        ).astype(f16)

        cores.append(
            {
                "ehT": ehT,
                "pk16": pk16,
                "pkf": pkf,
                "zr": zr,
                "aug": aug,
            }
        )
    return cores


def kernel(z, mu, embeddings, w1, b1, w2, b2, w3, b3):
    global LAST_RESULTS
    from concourse.bass_utils import run_bass_kernel_spmd

    _install_drain_patch()
    _install_wait_split_patch()
    if "nc" not in _CACHE:
        _CACHE["nc"] = _build_program()
    nc = _CACHE["nc"]

    in_maps = _host_pack(z, mu, embeddings, w1, b1, w2, b2, w3, b3)
    res = run_bass_kernel_spmd(nc, in_maps, list(range(_B)))
    LAST_RESULTS = res
    return np.stack(
        [res.results[c]["out"].astype(np.float32) for c in range(_B)], axis=0
    )
